# revision 5
# baseline (speedup 1.0000x reference)
"""MoE (top-1 routed) Trainium2 kernel.

Strategy: the reference computes every expert for every token and then
selects one expert per token with a one-hot gate.  Mathematically the
output for token n is expert_out[argmax_e logits[n, e], n], so we compute
the gating on host (bitwise-matching the reference's fp32 `x @ Wg + bg`
on CPU), group tokens by their selected expert, and run expert e's
pipeline for only its own tokens on NeuronCore e (expert-parallel, an
all-reduce-free gather).  This is 8x less device compute than the dense
reference formulation.

Device pipeline per core (C = padded token count, transposed layout with
features on partitions and tokens on the free dim):
    h^T[u, n]  = W1^T x^T          (PE, K=1024 accumulated in PSUM)
    sw         = (tanh(h/2) + 1) * h            # == 2*swish(h)
    z^T[v, n]  = (0.5*proj)^T sw   (PE)         # 0.5 folds the 2 above
    t2         = tanh(z/2)                      # == 2*sigmoid(z) - 1
    q          = exp((32/7) * t2)               # ONE exp per block
      -- the reference's gaussian basis times exp(32*xn^2) is
         g_j = exp(32*k_j*t2 + 32*k_j*(1-k_j)) = c_j * q^j  (k_j = j/7),
         so the basis numerator/denominator are degree-7 polynomials in
         q.  The per-element factor cancels in the normalization (the
         reference's +1e-6 in the denominator is a <=1.2e-6 relative
         perturbation, below fp32 matmul noise).
    powers q^2..q^7 via ACT square + DVE/GPSIMD multiplies
    num        = sum_j (cv_j*c_j) q^j   (PE: diagonal-matmul accumulation
                                         into PSUM; cv = ctrl * scaling)
    den        = 1 + sum_j c_j q^j      (PE: scaled-identity matmuls,
                                         the 1 via a ones tile)
    out^T[u,n] = (num + cv_0) * reciprocal(den)

tanh/exp/square share one ACT table set ("exp_and_others"), so the
scalar engine never pays the ~2.7us table switch.  swish(x) =
x*sigmoid(x) = 0.5*x*(1+tanh(x/2)) and sigmoid(z) = 0.5*(1+tanh(z/2))
are exact identities, with constants folded into proj / the exp args.

PACK_MODE="tile" packs the 128-wide diagonal matmuls as 4 concurrent
32x32 tile_position matmuls per step (the diagonal of a 128x128 diag
matrix only occupies the 4 diagonal subarrays; 4 chains with distinct
rotations fill 8-16 of the 16 subarray positions and run concurrently).
Chain outputs are then partition-block-rotated; the output DMA
unscrambles (2 descriptors per vc).
"""

import os
from contextlib import ExitStack

import numpy as np

N_TOK, D_IN, U_DIM, E_EXP, B_BAS = 8192, 1024, 512, 8, 8
N_CORES = 8
P = 128
TNMAX = 512

PACK_MODE = os.environ.get("MOE_PACK", "plain")  # "plain" | "tile"
N_PW_DVE = int(os.environ.get("MOE_PW_DVE", "2"))  # of the 4 TT powers, how many on DVE (rest GPSIMD)
G_BUFS = int(os.environ.get("MOE_GBUFS", "32"))
X_BUFS = int(os.environ.get("MOE_XBUFS", "2"))
PS_HZ_BUFS = int(os.environ.get("MOE_PS_HZ", "3"))
PS_ND_BUFS = int(os.environ.get("MOE_PS_ND", "4"))

_prog_cache = {}


def _knot_consts():
    # g_j = exp(32*k_j*t2 + 32*k_j*(1-k_j)) = c_j * q^j,  q = exp((32/7)*t2)
    ks = np.linspace(0.0, 1.0, B_BAS).astype(np.float64)
    cj = np.exp(32.0 * ks * (1.0 - ks))  # c_0 = c_7 = 1
    return ks, cj


def build_program(C, mm_mode, b1_zero):
    """Build + compile the SPMD single-core program for capacity C."""
    import concourse.tile as tile
    from concourse import bacc, mybir

    f32 = mybir.dt.float32
    f32r = mybir.dt.float32r
    add = mybir.AluOpType.add
    mult = mybir.AluOpType.mult
    Tanh = mybir.ActivationFunctionType.Tanh
    Exp = mybir.ActivationFunctionType.Exp
    Square = mybir.ActivationFunctionType.Square

    mm_dt = f32r

    assert C % P == 0
    tiles = []
    t0 = 0
    while C - t0 >= TNMAX:
        tiles.append((t0, TNMAX))
        t0 += TNMAX
    if C - t0 > 0:
        tiles.append((t0, C - t0))

    _, cj = _knot_consts()
    QS = 32.0 / 7.0  # exp scale

    nc = bacc.Bacc("TRN2", target_bir_lowering=False, debug=False,
                   num_devices=N_CORES)

    xT = nc.dram_tensor("xT", [D_IN, C], mm_dt, kind="ExternalInput").ap()
    w1 = nc.dram_tensor("w1", [D_IN, U_DIM], mm_dt, kind="ExternalInput").ap()
    p5 = nc.dram_tensor("p5", [U_DIM, U_DIM], mm_dt, kind="ExternalInput").ap()
    # auxn[vc*7+(j-1)] = diag(cv_j * c_j) for the vc'th 128-unit block
    auxn = nc.dram_tensor("auxn", [28, P, P], f32r, kind="ExternalInput").ap()
    # auxd[j] = c_j * I (j=1..7), auxd[7]... layout: [8,P,P]; auxd[0] = I (ones term)
    auxd = nc.dram_tensor("auxd", [8, P, P], f32r, kind="ExternalInput").ap()
    # a0 per unit (cv_0), layout [P, 4]
    a0h = nc.dram_tensor("a0h", [P, 4], f32, kind="ExternalInput").ap()
    onesd = nc.dram_tensor("onesd", [P, TNMAX], f32r, kind="ExternalInput").ap()
    b1h = nc.dram_tensor("b1h", [P, 4], f32, kind="ExternalInput").ap()
    outT = nc.dram_tensor("outT", [U_DIM, C], f32, kind="ExternalOutput").ap()

    xT_r = xT.rearrange("(kc p) c -> p kc c", p=P)
    auxn_r = auxn.rearrange("a p q -> p a q")
    auxd_r = auxd.rearrange("a p q -> p a q")
    w1_r = w1.rearrange("(kc p) u -> p kc u", p=P)
    p5_r = p5.rearrange("(uc p) v -> p uc v", p=P)
    outT_r = outT.rearrange("(vc p) c -> p vc c", p=P)

    with tile.TileContext(nc) as tc, ExitStack() as ctx:
        cpool = ctx.enter_context(tc.tile_pool(name="consts", bufs=1))
        xpool = ctx.enter_context(tc.tile_pool(name="x", bufs=X_BUFS))
        pshz = ctx.enter_context(tc.tile_pool(name="pshz", bufs=PS_HZ_BUFS,
                                              space="PSUM"))
        psnd = ctx.enter_context(tc.tile_pool(name="psnd", bufs=PS_ND_BUFS,
                                              space="PSUM"))
        epool = ctx.enter_context(tc.tile_pool(name="elem", bufs=4))
        swpool = ctx.enter_context(tc.tile_pool(name="sw", bufs=6))
        gpool = ctx.enter_context(tc.tile_pool(name="g", bufs=G_BUFS))
        mpool = ctx.enter_context(tc.tile_pool(name="m", bufs=4))
        opool = ctx.enter_context(tc.tile_pool(name="o", bufs=2))

        # x token tiles: issue ALL loads first so tile 0's data races the
        # (larger) weight loads instead of queueing behind them
        xq = []
        for (t0, TN) in tiles:
            xa = xpool.tile([P, 4, TNMAX], mm_dt, tag="xa", name=f"xa{t0}")
            nc.sync.dma_start(xa[:, :, :TN], xT_r[:, 0:4, t0:t0 + TN])
            xb = xpool.tile([P, 4, TNMAX], mm_dt, tag="xb", name=f"xb{t0}")
            nc.sync.dma_start(xb[:, :, :TN], xT_r[:, 4:8, t0:t0 + TN])
            xq.append((xa, xb))

        # resident weights on the ACT queue (parallel with x on sync)
        w1k = []
        for kc in range(8):
            t = cpool.tile([P, U_DIM], mm_dt, tag=f"w1_{kc}")
            nc.scalar.dma_start(t[:], w1_r[:, kc, :])
            w1k.append(t)
        puc = []
        for uc in range(4):
            t = cpool.tile([P, U_DIM], mm_dt, tag=f"p5_{uc}")
            eng = nc.sync if uc % 2 == 0 else nc.scalar
            eng.dma_start(t[:], p5_r[:, uc, :])
            puc.append(t)
        # small/late-needed constants via the gpsimd SWDGE queue
        auxnsb = cpool.tile([P, 28, P], mm_dt, tag="auxn")
        nc.gpsimd.dma_start(auxnsb[:], auxn_r[:])
        auxdsb = cpool.tile([P, 8, P], mm_dt, tag="auxd")
        nc.gpsimd.dma_start(auxdsb[:], auxd_r[:])
        a0sb = cpool.tile([P, 4], f32, tag="a0h")
        nc.gpsimd.dma_start(a0sb[:], a0h[:])
        ones = cpool.tile([P, TNMAX], mm_dt, tag="ones")
        nc.gpsimd.dma_start(ones[:], onesd[:])
        if not b1_zero:
            b1sb = cpool.tile([P, 4], f32, tag="b1h")
            nc.gpsimd.dma_start(b1sb[:], b1h[:])

        for ti, (t0, TN) in enumerate(tiles):
            xa, xb = xq[ti]

            # ---- h = x @ W1 ; sw = 2*swish(h) -----------------------
            sws = []
            for uc in range(4):
                hps = pshz.tile([P, TNMAX], f32, tag="ps", name="hps")
                for kc in range(8):
                    xt = xa if kc < 4 else xb
                    nc.tensor.matmul(
                        hps[:, :TN],
                        lhsT=w1k[kc][:, uc * P:(uc + 1) * P],
                        rhs=xt[:, kc % 4, :TN],
                        start=(kc == 0), stop=(kc == 7),
                    )
                th = epool.tile([P, TNMAX], f32, tag="th")
                if b1_zero:
                    nc.scalar.activation(th[:, :TN], hps[:, :TN], Tanh,
                                         scale=0.5)
                else:
                    nc.scalar.activation(th[:, :TN], hps[:, :TN], Tanh,
                                         scale=0.5, bias=b1sb[:, uc:uc + 1])
                sw = swpool.tile([P, TNMAX], mm_dt, tag="sw")
                if b1_zero:
                    # sw = (th + 1) * h  == 2*swish(h)
                    nc.vector.scalar_tensor_tensor(
                        sw[:, :TN], th[:, :TN], 1.0, hps[:, :TN],
                        op0=add, op1=mult)
                else:
                    y = epool.tile([P, TNMAX], f32, tag="y")
                    nc.vector.tensor_scalar(
                        y[:, :TN], hps[:, :TN], b1sb[:, uc:uc + 1], None,
                        op0=add)
                    nc.vector.scalar_tensor_tensor(
                        sw[:, :TN], th[:, :TN], 1.0, y[:, :TN],
                        op0=add, op1=mult)
                sws.append(sw)

            # ---- z = sw @ (0.5*proj); q powers ----------------------
            pw = []  # pw[vc] = [None, q, q2, ..., q7]
            for vc in range(4):
                zps = pshz.tile([P, TNMAX], f32, tag="ps", name="zps")
                for uc in range(4):
                    nc.tensor.matmul(
                        zps[:, :TN],
                        lhsT=puc[uc][:, vc * P:(vc + 1) * P],
                        rhs=sws[uc][:, :TN],
                        start=(uc == 0), stop=(uc == 3),
                    )
                t2 = epool.tile([P, TNMAX], f32, tag="t2")
                nc.scalar.activation(t2[:, :TN], zps[:, :TN], Tanh, scale=0.5)

                q = [None] * 8
                qf = [None] * 8
                for j in (1, 2, 3, 4, 5, 6, 7):
                    q[j] = gpool.tile([P, TNMAX], mm_dt, tag="g",
                                      name=f"q{j}_{vc}")
                    qf[j] = q[j].bitcast(f32)
                # q, q2, q4 on ACT (exp + 2 squares); q3,q5,q6,q7 as
                # tensor-tensor products split DVE/GPSIMD
                nc.scalar.activation(q[1][:, :TN], t2[:, :TN], Exp, scale=QS)
                nc.scalar.activation(q[2][:, :TN], qf[1][:, :TN], Square)
                nc.scalar.activation(q[4][:, :TN], qf[2][:, :TN], Square)
                tt_plan = [(3, 1, 2), (5, 1, 4), (6, 2, 4), (7, 3, 4)]
                for idx, (jo, ja, jb) in enumerate(tt_plan):
                    eng = nc.vector if idx < N_PW_DVE else nc.gpsimd
                    eng.tensor_tensor(q[jo][:, :TN], qf[ja][:, :TN],
                                      qf[jb][:, :TN], mult)
                pw.append(q)

            # ---- num/den accumulation on PE -------------------------
            outb = opool.tile([P, 4, TNMAX], f32, tag="outb")
            if PACK_MODE == "plain":
                for vc in range(4):
                    q = pw[vc]
                    nps = psnd.tile([P, TNMAX], f32, tag="ps", name="nps")
                    for j in range(1, 8):
                        nc.tensor.matmul(
                            nps[:, :TN],
                            lhsT=auxnsb[:, vc * 7 + (j - 1), :],
                            rhs=q[j][:, :TN],
                            start=(j == 1), stop=(j == 7))
                    dps = psnd.tile([P, TNMAX], f32, tag="ps", name="dps")
                    for j in range(8):
                        rhs = ones[:, :TN] if j == 0 else q[j][:, :TN]
                        nc.tensor.matmul(
                            dps[:, :TN],
                            lhsT=auxdsb[:, j, :],
                            rhs=rhs,
                            start=(j == 0), stop=(j == 7))
                    r = mpool.tile([P, TNMAX], f32, tag="r", name=f"r{vc}")
                    nc.vector.reciprocal_approx_fast(r[:, :TN], dps[:, :TN])
                    nc.vector.scalar_tensor_tensor(
                        outb[:, vc, :TN], nps[:, :TN], a0sb[:, vc:vc + 1],
                        r[:, :TN], op0=add, op1=mult)
                nc.sync.dma_start(outT_r[:, :, t0:t0 + TN], outb[:, :, :TN])
            else:
                # tile-packed: pairs (A,B) = (0,1), (2,3); rot(A)=0 rot(B)=1
                # round 1: numA (rot0) || denB (rot1)
                # round 2: denA (rot0) || numB (rot1)
                for (A, B) in ((0, 1), (2, 3)):
                    npsA = psnd.tile([P, TNMAX], f32, tag="ps", name="npsA")
                    dpsB = psnd.tile([P, TNMAX], f32, tag="ps", name="dpsB")
                    for j in range(8):
                        for rr in range(4):
                            if j >= 1:  # num has 7 terms (j=1..7)
                                cA = rr  # rot 0
                                nc.tensor.matmul(
                                    npsA[cA * 32:(cA + 1) * 32, :TN],
                                    lhsT=auxnsb[rr * 32:(rr + 1) * 32,
                                                A * 7 + (j - 1),
                                                cA * 32:(cA + 1) * 32],
                                    rhs=pw[A][j][rr * 32:(rr + 1) * 32, :TN],
                                    start=(j == 1), stop=(j == 7),
                                    tile_position=(rr * 32, cA * 32))
                            cB = (rr + 1) % 4  # rot 1
                            rhs = (ones if j == 0 else pw[B][j])
                            nc.tensor.matmul(
                                dpsB[cB * 32:(cB + 1) * 32, :TN],
                                lhsT=auxdsb[rr * 32:(rr + 1) * 32, j,
                                            cB * 32:(cB + 1) * 32],
                                rhs=rhs[rr * 32:(rr + 1) * 32, :TN],
                                start=(j == 0), stop=(j == 7),
                                tile_position=(rr * 32, cB * 32))
                    dpsA = psnd.tile([P, TNMAX], f32, tag="ps", name="dpsA")
                    npsB = psnd.tile([P, TNMAX], f32, tag="ps", name="npsB")
                    for j in range(8):
                        for rr in range(4):
                            cA = rr
                            rhs = (ones if j == 0 else pw[A][j])
                            nc.tensor.matmul(
                                dpsA[cA * 32:(cA + 1) * 32, :TN],
                                lhsT=auxdsb[rr * 32:(rr + 1) * 32, j,
                                            cA * 32:(cA + 1) * 32],
                                rhs=rhs[rr * 32:(rr + 1) * 32, :TN],
                                start=(j == 0), stop=(j == 7),
                                tile_position=(rr * 32, cA * 32))
                            if j >= 1:
                                cB = (rr + 1) % 4
                                nc.tensor.matmul(
                                    npsB[cB * 32:(cB + 1) * 32, :TN],
                                    lhsT=auxnsb[rr * 32:(rr + 1) * 32,
                                                B * 7 + (j - 1),
                                                cB * 32:(cB + 1) * 32],
                                    rhs=pw[B][j][rr * 32:(rr + 1) * 32, :TN],
                                    start=(j == 1), stop=(j == 7),
                                    tile_position=(rr * 32, cB * 32))
                    for vc, nps, dps in ((A, npsA, dpsA), (B, npsB, dpsB)):
                        r = mpool.tile([P, TNMAX], f32, tag="r",
                                       name=f"r{vc}")
                        nc.vector.reciprocal_approx_fast(r[:, :TN],
                                                         dps[:, :TN])
                        # a0 pre-rotated host-side to match rot(vc)
                        nc.vector.scalar_tensor_tensor(
                            outb[:, vc, :TN], nps[:, :TN],
                            a0sb[:, vc:vc + 1],
                            r[:, :TN], op0=add, op1=mult)
                # out DMA with rotation unscramble: SBUF partition block c
                # of vc holds units 32*((c - rot) % 4); rot = vc % 2
                for vc in range(4):
                    rot = vc % 2
                    if rot == 0:
                        nc.sync.dma_start(outT_r[:, vc, t0:t0 + TN],
                                          outb[:, vc, :TN])
                    else:
                        # SBUF partitions [32*rot, 128) -> unit rows [0, ..)
                        nr = (4 - rot) * 32
                        nc.sync.dma_start(
                            outT_r[0:nr, vc, t0:t0 + TN],
                            outb[rot * 32:128, vc, :TN])
                        nc.sync.dma_start(
                            outT_r[nr:128, vc, t0:t0 + TN],
                            outb[0:rot * 32, vc, :TN])

    nc.compile()
    return nc, tiles


def _get_program(C, mm_mode, b1_zero):
    key = (C, mm_mode, b1_zero, PACK_MODE, N_PW_DVE, G_BUFS, X_BUFS,
           PS_HZ_BUFS, PS_ND_BUFS)
    if key not in _prog_cache:
        _prog_cache[key] = build_program(C, mm_mode, b1_zero)
    return _prog_cache[key]


def _route_on_host(x, Wg, bg):
    """Expert assignment, bitwise-matching the reference's fp32 CPU math."""
    import jax
    import jax.numpy as jnp

    cpu = jax.devices("cpu")[0]
    with jax.default_device(cpu):
        logits = jnp.asarray(x) @ jnp.asarray(Wg) + jnp.asarray(bg)
        eid = np.asarray(jnp.argmax(logits, axis=-1))
    return eid


def make_in_maps(x, W1, b1, proj, ctrl, scaling, Wg, bg, mm_mode="f32r"):
    x = np.asarray(x, dtype=np.float32)
    eid = _route_on_host(x, Wg, bg)
    order = np.argsort(eid, kind="stable")
    counts = np.bincount(eid, minlength=E_EXP)
    starts = np.zeros(E_EXP + 1, dtype=np.int64)
    starts[1:] = np.cumsum(counts)
    C = int(max(counts.max(), 1))
    C = ((C + P - 1) // P) * P

    _, cj = _knot_consts()

    cvf = (np.asarray(ctrl, np.float32)
           * np.asarray(scaling, np.float32)[:, None, :])  # [E, B, U]
    proj5 = 0.5 * np.asarray(proj, np.float32)
    b1f = np.asarray(b1, np.float32)
    b1_zero = not np.any(b1f)

    packed = PACK_MODE == "tile"
    ar = np.arange(P)

    in_maps = []
    for e in range(E_EXP):
        idx = order[starts[e]:starts[e + 1]]
        xT = np.zeros((D_IN, C), dtype=np.float32)
        if len(idx):
            xT[:, :len(idx)] = x[idx].T
        b1h = np.ascontiguousarray(
            (0.5 * b1f[e]).reshape(4, P).T).astype(np.float32)
        # num weights: a_j[u] = cv_j[u] * c_j (j=1..7), per 128-unit block
        auxn = np.zeros((28, P, P), dtype=np.float32)
        for vc in range(4):
            for j in range(1, 8):
                w = cvf[e][j, vc * P:(vc + 1) * P] * cj[j]
                if packed:
                    rot = vc % 2
                    # output block c = (r + rot) % 4 holds units of block r:
                    # diag block at [rows 32r.., cols 32c..]
                    for rr in range(4):
                        cc = (rr + rot) % 4
                        a32 = np.arange(32)
                        auxn[vc * 7 + (j - 1),
                             rr * 32 + a32, cc * 32 + a32] = \
                            w[rr * 32 + a32]
                else:
                    auxn[vc * 7 + (j - 1), ar, ar] = w
        # den weights: c_j * I (j=1..7); j=0 slot = I (ones term). For
        # packed mode every 32-block diagonal position is filled so any
        # (r, c) block is diag(c_j).
        auxd = np.zeros((8, P, P), dtype=np.float32)
        for j in range(8):
            cval = 1.0 if j == 0 else cj[j]
            if packed:
                a32 = np.arange(32)
                for rr in range(4):
                    for cc2 in range(4):
                        auxd[j, rr * 32 + a32, cc2 * 32 + a32] = cval
            else:
                auxd[j, ar, ar] = cval
        # a0 = cv_0 per unit, layout [P, vc]; pre-rotated for packed mode
        a0 = np.zeros((P, 4), dtype=np.float32)
        for vc in range(4):
            v = cvf[e][0, vc * P:(vc + 1) * P]
            if packed:
                rot = vc % 2
                vr = v.reshape(4, 32)
                a0[:, vc] = np.roll(vr, rot, axis=0).reshape(P)
            else:
                a0[:, vc] = v
        in_maps.append({
            "xT": xT,
            "w1": np.asarray(W1[e], np.float32),
            "p5": proj5[e],
            "auxn": auxn,
            "auxd": auxd,
            "a0h": a0,
            "b1h": b1h,
            "onesd": np.ones((P, TNMAX), dtype=np.float32),
        })
    return in_maps, order, starts, counts, C, b1_zero


def kernel(x, W1, b1, proj, ctrl, scaling, Wg, bg):
    from concourse.bass_utils import run_bass_kernel_spmd

    in_maps, order, starts, counts, C, b1_zero = make_in_maps(
        x, W1, b1, proj, ctrl, scaling, Wg, bg)
    nc, _ = _get_program(C, "f32r", b1_zero)

    res = run_bass_kernel_spmd(nc, in_maps, list(range(N_CORES)))

    out = np.empty((N_TOK, U_DIM), dtype=np.float32)
    for e in range(E_EXP):
        cnt = int(counts[e])
        if cnt:
            out[order[starts[e]:starts[e + 1]]] = \
                res.results[e]["outT"][:, :cnt].T
    return out


MM_MODE = "f32r"  # kept for test.py compatibility
